# revision 15
# baseline (speedup 1.0000x reference)
"""Trainium2 Bass kernel for nn_MultiHeadAttention_70866960384614.

MHA: B=2, S=2048, D_MODEL=1024, HEADS=16, D_K=64, softmax(|QK^T|/8) @ V.

Sharding (8 cores): data-parallel over batch (2) x tensor-parallel over head
groups (4 groups of 4 heads). Host pre-transposes x and the weight slices so
the device does ZERO layout transposes: per core the inputs are
  xqT/xkT/xvT [1024, 2048]  (x^T, bf16)
  wqT/wkT/wvT [1024, 256]   (W[group].T, bf16)
  woT         [256, 1024]   (Wo[:, group].T, bf16)
  bqt/bkt     [128, 2]      (K/Q bias, f32, partition=channel%128, col=ch//128)

Per-core dataflow (v2 — elementwise-balanced, transposed PV):
  QT = Wq @ xT            [256, 2048] f32r  (bias folded into the PSUM exit)
  KT likewise; V = x @ WvT stored kv-major bf16 with a ones column per head
  per (head-pair pr, q-chunk of 512): for each kv pair:
     ST = K Q^T (2 heads in the PE array) -> PSUM
     abs -> sabs f32 SBUF   (DVE bitwise / ACT AF.Abs, load-balanced)
     exp -> pa bf16 SBUF    (one ACT op per kv-PAIR [128,2048])
     PV (transposed): out[q,65] += P_block^T-free matmuls: stationary =
       pa [128kv,128q] bf16, moving = V|1 [128kv,65] bf16 -> 65-row matmuls
       (half the PE rows of the [65,512]-orientation PV)
  tail: 1/Z via DVE reciprocal of the PSUM ones-column, per-partition mult
        exits -> cat bf16 [q, ch]; DMA-transpose -> catT [ch, q];
        out partial = catT.T @ WoT -> DMA (host sums 4 group partials + bias)
"""

import json
from collections import deque
from contextlib import ExitStack

import numpy as np

import concourse.bass as bass
import concourse.mybir as mybir
import concourse.tile as tile
from concourse.vector_clock import ScopedClock

F32 = mybir.dt.float32
F32R = mybir.dt.float32r
BF16 = mybir.dt.bfloat16
U32 = mybir.dt.uint32
AF = mybir.ActivationFunctionType
ALU = mybir.AluOpType

S = 2048
D = 1024
HG = 4            # heads per core
DK = 64
GC = HG * DK      # 256
P = 128
N_CORES = 8
SC = 512          # s-chunk for projection streaming
QC = 512          # q-chunk within attention
KVB = S // P      # 16 kv blocks
SCALE = 0.125
PV_LAG = 3        # pair-batches of PV kept pending behind the ST stream


class TileContextCompat(tile.TileContext):
    """This container's walrus build rejects >1 sync-wait on a CTRL (Drain)
    instruction; spread the kernel-tail DMA-lane waits across one drain
    each instead of piling them on a single drain."""

    def _drain_and_barrier(self, tick_clock, wait_clock):
        drain_inst = self.nc.sync.drain()
        wait_clock.add_sem_waits(
            drain_inst.ins, ScopedClock({None: tick_clock.global_clock}))
        si = drain_inst.ins.sync_info
        extra = []
        if si is not None and si.on_wait is not None:
            while len(si.on_wait) > 1:
                extra.append(si.on_wait.pop())
        for w in extra:
            d2 = self.nc.sync.drain()
            if d2.ins.sync_info is None:
                d2.ins.sync_info = mybir.SyncInfo(on_wait=[w], on_update=[])
            else:
                d2.ins.sync_info.on_wait.append(w)
        self.nc.all_engine_barrier()
        assert self.sems is not None
        popped = self.nc._tile_sem_poison_stack.pop()
        assert popped is self._sem_poison
        self.nc.clear_and_free_semaphores(list(self.sems.allocated().values()))
        self.nc.all_engine_barrier()


def build_nc():
    nc = bass.Bass("TRN2", target_bir_lowering=False, debug=False,
                   num_devices=N_CORES)

    xqt = nc.dram_tensor("xqt", [D, S], BF16, kind="ExternalInput").ap()
    xkt = nc.dram_tensor("xkt", [D, S], BF16, kind="ExternalInput").ap()
    xvt = nc.dram_tensor("xvt", [D, S], BF16, kind="ExternalInput").ap()
    wqt = nc.dram_tensor("wqt", [D, GC], BF16, kind="ExternalInput").ap()
    wkt = nc.dram_tensor("wkt", [D, GC], BF16, kind="ExternalInput").ap()
    wvt = nc.dram_tensor("wvt", [D, GC], BF16, kind="ExternalInput").ap()
    wot = nc.dram_tensor("wot", [GC, D], BF16, kind="ExternalInput").ap()
    bqt = nc.dram_tensor("bqt", [P, 2], F32, kind="ExternalInput").ap()
    bkt = nc.dram_tensor("bkt", [P, 2], F32, kind="ExternalInput").ap()
    out = nc.dram_tensor("out", [S, D], F32, kind="ExternalOutput").ap()

    with ExitStack() as ctx:
        tc = ctx.enter_context(TileContextCompat(nc))
        _emit(ctx, tc, xqt, xkt, xvt, wqt, wkt, wvt, wot, bqt, bkt, out)

    fixed = _split_multi_waits(nc.to_json_bytes())
    nc.to_json_bytes = lambda: fixed
    return nc


def _split_multi_waits(raw):
    """Walrus here accepts only one sync-wait per instruction; hoist extras
    onto wait-only EventSemaphore instructions on the same engine."""
    m = json.loads(raw)
    counter = [0]

    def fix_block(b):
        new = []
        for inst in b.get("instructions", []):
            si = inst.get("sync_info")
            if si and si.get("on_wait") and len(si["on_wait"]) > 1:
                waits = si["on_wait"]
                for w in waits[:-1]:
                    counter[0] += 1
                    new.append({
                        "debug": inst.get("debug", 0),
                        "engine": inst["engine"],
                        "ins": [],
                        "outs": [],
                        "name": f"I-wsplit-{counter[0]}",
                        "opcode": "EventSemaphore",
                        "sync_info": {"on_update": [], "on_wait": [w]},
                    })
                si["on_wait"] = waits[-1:]
            new.append(inst)
        b["instructions"] = new
        for sub in b.get("blocks", []):
            fix_block(sub)

    for fn in m["functions"]:
        for b in fn.get("blocks", []):
            fix_block(b)
    return json.dumps(m).encode()


def _emit(ctx, tc, xqt, xkt, xvt, wqt, wkt, wvt, wot, bqt, bkt, out):
    nc = tc.nc

    persist = ctx.enter_context(tc.tile_pool(name="persist", bufs=1))
    xs = ctx.enter_context(tc.tile_pool(name="xs", bufs=3))
    stp = ctx.enter_context(tc.tile_pool(name="st", bufs=2, space="PSUM"))
    opp = ctx.enter_context(tc.tile_pool(name="op", bufs=4, space="PSUM"))
    sap = ctx.enter_context(tc.tile_pool(name="sa", bufs=3))
    pap = ctx.enter_context(tc.tile_pool(name="pa", bufs=PV_LAG + 1))
    cbp = ctx.enter_context(tc.tile_pool(name="cb", bufs=2))
    ctp = ctx.enter_context(tc.tile_pool(name="ct", bufs=2))
    rcp = ctx.enter_context(tc.tile_pool(name="rc", bufs=2))
    otp = ctx.enter_context(tc.tile_pool(name="ot", bufs=4))

    # ------------------------------------------------------- persistent
    qT = persist.tile([P, 2, S], F32R)
    kT = persist.tile([P, 2, S], F32R)
    vA = persist.tile([P, KVB, HG * (DK + 1)], BF16)
    wq_s = persist.tile([P, D // P, GC], BF16)
    wk_s = persist.tile([P, D // P, GC], BF16)
    wv_s = persist.tile([P, D // P, GC], BF16)
    wo_s = persist.tile([P, GC // P, D], BF16)
    bq_r = persist.tile([P, 2], F32)
    bk_r = persist.tile([P, 2], F32)

    ones_row = persist.tile([1, QC], BF16)
    nc.vector.memset(ones_row, 1.0)
    # V ones columns: memset everything to 1, V exits overwrite the :DK parts
    nc.vector.memset(vA, 1.0)

    # --------------------------------------------- engine load balancing
    # Elementwise PSUM exits can only run on DVE or ACT (GPSIMD cannot touch
    # PSUM).  Greedily assign each op to the engine with the lower projected
    # load; exp is ACT-only and dominates, so abs mostly lands on DVE.
    eload = {"dve": 0.0, "act": 0.0}

    def pick(dve_cost, act_cost):
        if eload["dve"] + dve_cost <= eload["act"] + act_cost:
            eload["dve"] += dve_cost
            return "dve"
        eload["act"] += act_cost
        return "act"

    def bal_copy(dst, src, dve_cost, act_cost):
        if pick(dve_cost, act_cost) == "dve":
            nc.vector.tensor_copy(dst, src)
        else:
            nc.scalar.activation(dst, src, AF.Copy)

    # ------------------------------------------------------ weight DMAs
    def dma_w(w_dram, w_sb):
        nc.sync.dma_start(
            w_sb, w_dram.rearrange("(kc p) c -> p kc c", p=P))

    dma_w(wkt, wk_s)
    dma_w(wqt, wq_s)
    nc.gpsimd.dma_start(bq_r, bqt)
    nc.gpsimd.dma_start(bk_r, bkt)

    # PE warmup: a continuous chain of tiny matmuls spans the initial DMA
    # window so the PE p-state is fully ramped when the projections start.
    wup = stp.tile([P, 2 * QC], F32, tag="st", name="wup")
    for _ in range(150):
        nc.tensor.matmul(wup[0:1, 0:DK], ones_row[0:1, 0:1],
                         ones_row[0:1, 0:DK], start=True, stop=True)

    def proj_dma(x_dram, sc):
        xt = xs.tile([P, D // P, SC], BF16, tag="xs")
        src = x_dram.rearrange("(kc p) s -> p kc s", p=P)[
            :, :, sc * SC:(sc + 1) * SC]
        nc.sync.dma_start(xt, src)
        return xt

    def proj_mm(xt, sc, which):
        """Matmuls+exit for one SC-chunk of a projection. which: 'k'|'v'|'q'.
        K/Q: bias is folded into the per-half PSUM exits (per-partition
        scalar add) instead of a ones-row matmul."""
        ps = stp.tile([P, 2 * QC], F32, tag="st", name="pj")
        if which in ("k", "q"):
            w_sb, dstT, b_r = ((wk_s, kT, bk_r) if which == "k"
                               else (wq_s, qT, bq_r))
            for m in range(2):
                half = ps[:, m * SC:(m + 1) * SC]
                for kc in range(D // P):
                    nc.tensor.matmul(
                        half, w_sb[:, kc, m * P:(m + 1) * P],
                        xt[:, kc, :], start=(kc == 0), stop=(kc == D // P - 1))
                dst = dstT[:, m, sc * SC:(sc + 1) * SC]
                if pick(800, 650) == "dve":
                    nc.vector.tensor_scalar(
                        dst, half, b_r[:, m:m + 1], None, ALU.add)
                else:
                    nc.scalar.activation(
                        dst, half, AF.Identity, bias=b_r[:, m:m + 1])
        else:
            for sb in range(SC // P):
                seg = ps[:, sb * GC:(sb + 1) * GC]
                for kc in range(D // P):
                    nc.tensor.matmul(
                        seg, xt[:, kc, sb * P:(sb + 1) * P],
                        wv_s[:, kc, :], start=(kc == 0),
                        stop=(kc == D // P - 1))
            gsb = sc * (SC // P)
            dstv = vA[:, gsb:gsb + 4, :].rearrange(
                "p s (h c) -> p s h c", h=HG)[:, :, :, :DK]
            srcv = ps[:].rearrange("p (s h c) -> p s h c", s=4, h=HG)
            # 3-free-dim AP: keep on ACT (walrus-proven shape)
            nc.scalar.activation(dstv, srcv, AF.Copy)
            eload["act"] += 1060

    def proj_chunk(x_dram, sc, which):
        proj_mm(proj_dma(x_dram, sc), sc, which)

    # K, Q chunk 0 first (unblocks the first ST pair), then V weights +
    # chunk 0. The remaining K/V chunks interleave into the qc0 stream.
    proj_chunk(xkt, 0, "k")
    proj_chunk(xqt, 0, "q")
    dma_w(wvt, wv_s)
    proj_chunk(xvt, 0, "v")

    # ---------------------------------------------------- attention
    pending_pv = deque()          # one closure per kv-pair (16 PV matmuls)
    pv_appended = [0]
    pv_flushed = [0]
    pending_tail = deque()        # (pv_barrier, urgent, closure)

    def flush_pv(keep=PV_LAG):
        while pending_pv and len(pending_pv) > keep:
            pending_pv.popleft()()
            pv_flushed[0] += 1

    def flush_tail(n=2):
        """Emit deferred tail work. `urgent` items (opp-slot readers) always
        drain fully once their pv barrier passed — they MUST be emitted
        before a later flush_pv rotates fresh PV accumulation onto their
        opp slots; relaxed items are metered at `n` per call."""
        done = 0
        while pending_tail and pending_tail[0][0] <= pv_flushed[0]:
            _, urgent, fn = pending_tail[0]
            if not urgent and done >= n:
                break
            pending_tail.popleft()
            fn()
            if not urgent:
                done += 1

    opp_tiles = {}   # (pr) -> current (oppA, oppB) accumulators
    cat_tiles = {}   # pr -> cat_b tile for the in-flight block
    catT_tiles = {}  # qc -> catT tile

    def get_catT(qc):
        if qc not in catT_tiles:
            catT_tiles[qc] = ctp.tile([P, 4, 2, P], BF16, tag="ct",
                                      name=f"catT{qc}")
        return catT_tiles[qc]

    def emit_pair(qc, pr, p):
        """One kv-pair of the score stream for (head-pair pr, q-chunk qc)."""
        flush_tail()
        flush_pv()
        if p == 0:
            opp_tiles[pr] = (
                opp.tile([P, 4, P], F32, tag="o", name=f"oA{pr}"),
                opp.tile([P, 4, P], F32, tag="o", name=f"oB{pr}"))
        oA, oB = opp_tiles[pr]
        qsl = slice(qc * QC, (qc + 1) * QC)
        sabs = sap.tile([P, 2 * 2 * QC], F32, tag="sa")
        for i in range(2):
            kv = 2 * p + i
            ksl = slice(kv * P, (kv + 1) * P)
            st = stp.tile([P, 2 * QC], F32, tag="st", name="stt")
            nc.tensor.matmul(
                st[:, :QC], kT[0:DK, pr, ksl],
                qT[0:DK, pr, qsl], start=True, stop=True,
                tile_position=(0, 0))
            nc.tensor.matmul(
                st[:, QC:], kT[DK:P, pr, ksl],
                qT[DK:P, pr, qsl], start=True, stop=True,
                tile_position=(DK, 0))
            dst = sabs[:, i * 2 * QC:(i + 1) * 2 * QC]
            if pick(1360, 1060) == "dve":
                nc.vector.tensor_scalar(
                    dst.bitcast(U32), st.bitcast(U32), 0x7FFFFFFF, None,
                    ALU.bitwise_and)
            else:
                nc.scalar.activation(dst, st, AF.Abs)
        flush_tail()
        pa = pap.tile([P, 2 * 2 * QC], BF16, tag="pa")
        nc.scalar.activation(pa, sabs, AF.Exp, scale=SCALE)
        eload["act"] += 1900

        def mk_pv(p=p, pa=pa, oA=oA, oB=oB):
            for i in range(2):
                kv = 2 * p + i
                for hh, ot in ((0, oA), (1, oB)):
                    h = 2 * pr + hh
                    vsl = vA[:, kv, h * (DK + 1):(h + 1) * (DK + 1)]
                    for qb in range(4):
                        off = i * 2 * QC + hh * QC + qb * P
                        # start=True zeroes the WHOLE bank and aborts any
                        # open accumulation group in it: exactly one start
                        # per opp tile (the 4 qb groups interleave per kv).
                        nc.tensor.matmul(
                            ot[:, qb, 0:DK + 1], pa[:, off:off + P],
                            vsl, start=(kv == 0 and qb == 0),
                            stop=(kv == KVB - 1), skip_group_check=True)
        pending_pv.append(mk_pv)
        pv_appended[0] += 1

    def block_tails(qc, pr, proj_tail=True):
        """Normalization + transpose tails for a finished (pr, qc) block."""
        barrier = pv_appended[0]
        oA, oB = opp_tiles[pr]

        def t_norm_a(qc=qc, pr=pr, oA=oA, oB=oB):
            rec = rcp.tile([P, 2, 4], F32, tag=f"rc{pr}", name=f"rc{pr}")
            nc.vector.reciprocal(rec[:, 0, :], oA[:, :, DK])
            nc.vector.reciprocal(rec[:, 1, :], oB[:, :, DK])
            eload["dve"] += 460
            cat_tiles[pr] = (rec, cbp.tile([P, 4, P], BF16, tag=f"cb{pr}",
                                           name=f"cb{pr}"))

        def t_norm_b(pr=pr, oA=oA, oB=oB, hh=0):
            rec, cat_b = cat_tiles[pr]
            ot = oA if hh == 0 else oB
            for qb in range(4):
                dst = cat_b[:, qb, hh * DK:(hh + 1) * DK]
                if pick(270, 245) == "dve":
                    nc.vector.tensor_scalar(
                        dst, ot[:, qb, 0:DK], rec[:, hh, qb:qb + 1],
                        None, ALU.mult)
                else:
                    nc.scalar.activation(
                        dst, ot[:, qb, 0:DK], AF.Copy,
                        scale=rec[:, hh, qb:qb + 1])

        def t_transpose(qc=qc, pr=pr):
            _, cat_b = cat_tiles[pr]
            catT = get_catT(qc)
            for j in range(4):
                nc.sync.dma_start_transpose(catT[:, j, pr, :], cat_b[:, j, :])

        pending_tail.append((barrier, True, t_norm_a))
        pending_tail.append((barrier, True, lambda: t_norm_b(hh=0)))
        pending_tail.append((barrier, True, lambda: t_norm_b(hh=1)))
        pending_tail.append((barrier, False, t_transpose))
        if pr == 0:
            if proj_tail and qc + 1 < S // QC:
                pending_tail.append(
                    (barrier, False,
                     lambda qc=qc: proj_mm(xq_box[0], qc + 1, "q")))
        else:
            for j in range(4):
                pending_tail.append(
                    (barrier, False, lambda qc=qc, j=j: outproj(qc, j)))

    def outproj(qc, j):
        catT = get_catT(qc)
        sb = qc * (QC // P) + j
        last = sb == S // P - 1
        o_t = otp.tile([P, D], F32, tag="ot")
        po = stp.tile([P, 2 * QC], F32, tag="st", name="po")
        for nn in range(2):
            seg = po[:, nn * QC:(nn + 1) * QC]
            for kc in range(2):
                nc.tensor.matmul(
                    seg, catT[:, j, kc, :],
                    wo_s[:, kc, nn * QC:(nn + 1) * QC],
                    start=(kc == 0), stop=(kc == 1))
        if last:
            # split exit+DMA per half so the closing DMA starts as soon as
            # the first half exits
            for h in range(2):
                hs = slice(h * QC, (h + 1) * QC)
                if h == 0:
                    nc.scalar.activation(o_t[:, hs], po[:, hs], AF.Copy)
                else:
                    nc.vector.tensor_copy(o_t[:, hs], po[:, hs])
                nc.sync.dma_start(out[sb * P:(sb + 1) * P, hs], o_t[:, hs])
        else:
            bal_copy(o_t, po, 1360, 1060)
            nc.sync.dma_start(out[sb * P:(sb + 1) * P, :], o_t)

    # --- qc 0: pr0/pr1 pair streams merged so the remaining K/V projection
    # chunks ride the attention stream (chunk sc covers kv blocks 4sc..4sc+3
    # = pairs 2sc..2sc+1, so chunk sc must be ready before pair 2sc).
    kv_tiles = {}
    xq1_tile = None
    for p in range(KVB // 2):
        if p in (0, 2, 4):
            sc = p // 2 + 1
            kv_tiles[sc] = [proj_dma(xkt, sc)]
        if p in (1, 3, 5):
            sc = (p + 1) // 2
            kv_tiles[sc].append(proj_dma(xvt, sc))
        if p in (2, 4, 6):
            sc = p // 2
            xk_t, xv_t = kv_tiles.pop(sc)
            proj_mm(xk_t, sc, "k")
            proj_mm(xv_t, sc, "v")
        emit_pair(0, 0, p)
        emit_pair(0, 1, p)
        if p == 5:
            nc.sync.dma_start(
                wo_s, wot.rearrange("(kc p) d -> p kc d", p=P))
        if p == 6:
            xq1_tile = proj_dma(xqt, 1)
        if p == 7:
            proj_mm(xq1_tile, 1, "q")
    block_tails(0, 0, proj_tail=False)
    block_tails(0, 1)

    # --- qc 1..3: per-(qc, pr) blocks with deferred tails. The next qc's
    # xq chunk DMA is issued eagerly at pr1 start; its matmuls ride the
    # pr0 tail so the data is on-chip well before the proj_mm lands.
    xq_box = [None]
    for qc in range(1, S // QC):
        for pr in range(2):
            if pr == 1 and qc + 1 < S // QC:
                xq_box[0] = proj_dma(xqt, qc + 1)
            for p in range(KVB // 2):
                emit_pair(qc, pr, p)
            block_tails(qc, pr)
    flush_pv(keep=0)
    while pending_tail:
        pending_tail.popleft()[2]()


_NC_CACHE = {}


def _get_nc():
    if "nc" not in _NC_CACHE:
        _NC_CACHE["nc"] = build_nc()
    return _NC_CACHE["nc"]


def make_in_maps(q, k, v, Wq, bq, Wk, bk, Wv, bv, Wo, bo):
    import ml_dtypes
    bf16 = ml_dtypes.bfloat16
    xT = [np.ascontiguousarray(np.asarray(a, np.float32).T.astype(bf16))
          for a in (q[0], k[0], v[0], q[1], k[1], v[1])]
    in_maps = []
    for c in range(N_CORES):
        b, g = divmod(c, 4)
        sl = slice(g * GC, (g + 1) * GC)
        in_maps.append({
            "xqt": xT[3 * b + 0],
            "xkt": xT[3 * b + 1],
            "xvt": xT[3 * b + 2],
            "wqt": np.ascontiguousarray(Wq[sl].T.astype(bf16)),
            "wkt": np.ascontiguousarray(Wk[sl].T.astype(bf16)),
            "wvt": np.ascontiguousarray(Wv[sl].T.astype(bf16)),
            "wot": np.ascontiguousarray(Wo[:, sl].T.astype(bf16)),
            "bqt": np.ascontiguousarray(
                bq[sl].reshape(2, P).T.astype(np.float32)),
            "bkt": np.ascontiguousarray(
                bk[sl].reshape(2, P).T.astype(np.float32)),
        })
    return in_maps


def kernel(q, k, v, Wq, bq, Wk, bk, Wv, bv, Wo, bo, _trace=False):
    from concourse.bass_utils import run_bass_kernel_spmd

    q, k, v = (np.asarray(a, np.float32) for a in (q, k, v))
    Wq, bq, Wk, bk, Wv, bv, Wo, bo = (
        np.asarray(a, np.float32) for a in (Wq, bq, Wk, bk, Wv, bv, Wo, bo))

    nc = _get_nc()
    in_maps = make_in_maps(q, k, v, Wq, bq, Wk, bk, Wv, bv, Wo, bo)
    res = run_bass_kernel_spmd(nc, in_maps, core_ids=list(range(N_CORES)),
                               trace=_trace)
    partials = np.stack([r["out"] for r in res.results])  # [8, S, D]
    # softmax rows sum to 1, so the V bias passes through attention exactly:
    # out += Wo @ bv (folded here) + bo
    bias = (Wo @ bv + bo).astype(np.float32)
    full = partials.reshape(2, 4, S, D).sum(axis=1) + bias[None, None, :]
    if _trace:
        return full.astype(np.float32), res
    return full.astype(np.float32)
